# revision 20
# baseline (speedup 1.0000x reference)
"""EnhancedLSTMCell Trainium2 kernel.

Data-parallel over 8 NeuronCores: batch B=8192 split into 8 shards of 1024
rows. Per core:
    gates = [x | h_prev] @ W + b          # [1024, 4096] via PE, fp32r
    i,f,g,o = split(gates); f *= mask
    c = f*c_prev + i*g; c = LayerNorm(c)*gamma + beta; h = o*tanh(c)

Layout: batch rows on partitions (8 chunks of 128), contraction dim K=2048 on
partitions for matmul operands. Activations are transposed on the PE
(identity matmul) into a resident combT [K=2048, 1024] SBUF buffer; W is
streamed once in 16 column-slices of 256. Bias is injected into PSUM with a
K=1 ones-row matmul so ACT engines can consume gate pre-activations straight
from PSUM. c accumulates in SBUF: i-drain writes sigmoid(i) in place, the
g-drain multiplies tanh(g) in, the f-drain adds (sigmoid(f)*mask)*c_prev.
LayerNorm uses bn_stats/bn_aggr + Sqrt/reciprocal; tanh(c_t) overwrites the
accumulator to feed h = sigmoid(o) * tanh(c_t).
"""

import sys

if "/opt/trn_rl_repo" not in sys.path:
    sys.path.insert(0, "/opt/trn_rl_repo")

import numpy as np

B = 8192
IN = 1024
H = 1024
NCORES = 8
BC = B // NCORES          # 1024 rows per core
MCH = BC // 128           # 8 partition chunks of batch rows
KCH = (IN + H) // 128     # 16 contraction chunks
CB = 256                  # W column-block width
EPS = 1e-5

_PROGRAMS = {}


def _build_program(trivial_gb: bool):
    from contextlib import ExitStack

    import concourse.bass as bass
    import concourse.tile as tile
    from concourse import bacc, mybir

    F32 = mybir.dt.float32
    F32R = mybir.dt.float32r
    AF = mybir.ActivationFunctionType
    ALU = mybir.AluOpType

    nc = bacc.Bacc("TRN2", target_bir_lowering=False, debug=False)

    # combined^T = [x | h_prev]^T per shard, transposed host-side during
    # sharding so the contraction dim lands on partitions with unit-stride DMA
    ct_d = nc.dram_tensor("combT", [IN + H, BC], F32, kind="ExternalInput").ap()
    c_d = nc.dram_tensor("c_prev", [BC, H], F32, kind="ExternalInput").ap()
    m_d = nc.dram_tensor("forget_mask", [MCH, 128], F32, kind="ExternalInput").ap()
    w_d = nc.dram_tensor("W", [IN + H, 4 * H], F32, kind="ExternalInput").ap()
    b_d = nc.dram_tensor("b", [1, 4 * H], F32, kind="ExternalInput").ap()
    g_d = nc.dram_tensor("ln_gamma", [1, H], F32, kind="ExternalInput").ap()
    be_d = nc.dram_tensor("ln_beta", [1, H], F32, kind="ExternalInput").ap()
    ho_d = nc.dram_tensor("h_out", [BC, H], F32, kind="ExternalOutput").ap()
    co_d = nc.dram_tensor("c_out", [BC, H], F32, kind="ExternalOutput").ap()

    w_k = w_d.rearrange("(k p) n -> p k n", p=128)  # [128, 16, 4096]
    ct_k = ct_d.rearrange("(k p) b -> p k b", p=128)  # [128, 16, 1024]

    with tile.TileContext(nc) as tc, ExitStack() as ctx:
        singles = ctx.enter_context(tc.tile_pool(name="singles", bufs=1))
        bigs = ctx.enter_context(tc.tile_pool(name="bigs", bufs=1))
        wpool = ctx.enter_context(tc.tile_pool(name="w", bufs=1))
        wrpool = ctx.enter_context(tc.tile_pool(name="wr", bufs=2))
        ctpool = ctx.enter_context(tc.tile_pool(name="ctstage", bufs=1))
        tpool = ctx.enter_context(tc.tile_pool(name="tmp", bufs=3))
        cppool = ctx.enter_context(tc.tile_pool(name="cprev", bufs=2))
        hpool = ctx.enter_context(tc.tile_pool(name="hout", bufs=2))
        zpool = ctx.enter_context(tc.tile_pool(name="z", bufs=2))
        pmain = ctx.enter_context(tc.tile_pool(name="pmain", bufs=6, space="PSUM"))

        # bias broadcast to all partitions (added to PSUM by DVE post-matmul)
        b_bc = singles.tile([128, 4 * H], F32)
        nc.sync.dma_start(
            out=b_bc,
            in_=bass.AP(tensor=b_d.tensor, offset=b_d.offset,
                        ap=[[0, 128], b_d.ap[1]]),
        )
        mask_sb = singles.tile([128, MCH], F32)
        nc.sync.dma_start(out=mask_sb, in_=m_d.rearrange("m p -> p m"))
        if not trivial_gb:
            gam_bc = singles.tile([128, H], F32)
            nc.sync.dma_start(
                out=gam_bc,
                in_=bass.AP(tensor=g_d.tensor, offset=g_d.offset,
                            ap=[[0, 128], g_d.ap[1]]),
            )
            bet_bc = singles.tile([128, H], F32)
            nc.sync.dma_start(
                out=bet_bc,
                in_=bass.AP(tensor=be_d.tensor, offset=be_d.offset,
                            ap=[[0, 128], be_d.ap[1]]),
            )

        # combT[k, m] = (128x128 transposed block of [x | h_prev]), stored
        # pre-rounded to fp32r for the PE
        combT = bigs.tile([128, KCH, MCH, 128], F32R)
        c_acc = bigs.tile([128, MCH, H], F32)
        mvall = singles.tile([128, MCH, 2], F32)
        std_t = singles.tile([128, MCH], F32)
        inv_t = singles.tile([128, MCH], F32)
        nmi_t = singles.tile([128, MCH], F32)
        eps_t = singles.tile([128, 1], F32)
        nc.vector.memset(eps_t, EPS)

        # ---- load combined^T and round to fp32r ----
        # Conversion runs on ACT so that PE matmuls only ever wait on the ACT
        # semaphore (wr conversion and psum drains are ACT-owned too): the
        # PE-LDW instruction class accepts only ONE sync wait.
        for m in range(MCH):
            cts = ctpool.tile([128, KCH, 128], F32, tag="cts")
            nc.gpsimd.dma_start(out=cts, in_=ct_k[:, :, m * 128:(m + 1) * 128])
            nc.scalar.copy(combT[:, :, m, :], cts)

        # ---- main loop over W column blocks ----
        # order: (i,g) interleaved per quarter, then f, then LN, then o
        GOFF = {"i": 0, "f": H, "g": 2 * H, "o": 3 * H}
        NQ = H // CB  # quarters per gate
        blocks = []
        for q in range(NQ):
            blocks.append(("i", q))
            blocks.append(("g", q))
        blocks += [("f", q) for q in range(NQ)]
        o_blocks = [("o", q) for q in range(NQ)]

        def do_block(gate, q):
            col0 = GOFF[gate] + q * CB
            wt = wpool.tile([128, KCH, CB], F32, tag="w")
            nc.gpsimd.dma_start(out=wt, in_=w_k[:, :, col0:col0 + CB])
            # fp32 -> fp32r rounding on ACT so PE matmuls only ever wait on
            # the ACT semaphore (psum-free is also ACT-owned)
            wr = wrpool.tile([128, KCH, CB], F32R, tag="wr")
            nc.scalar.copy(wr, wt)
            for m in range(MCH):
                ps = pmain.tile([128, CB], F32, tag="ps")
                for k in range(KCH):
                    nc.tensor.matmul(
                        ps, combT[:, k, m, :], wr[:, k, :],
                        start=(k == 0), stop=(k == KCH - 1),
                    )
                nc.vector.tensor_add(ps, ps, b_bc[:, col0:col0 + CB])
                csl = c_acc[:, m, q * CB:(q + 1) * CB]
                if gate == "i":
                    nc.scalar.activation(csl, ps, AF.Sigmoid)
                elif gate == "g":
                    tg = tpool.tile([128, CB], F32, tag="t")
                    nc.scalar.activation(tg, ps, AF.Tanh)
                    nc.vector.tensor_mul(csl, csl, tg)
                elif gate == "f":
                    tf = tpool.tile([128, CB], F32, tag="t")
                    nc.scalar.activation(tf, ps, AF.Sigmoid)
                    cp = cppool.tile([128, CB], F32, tag="cp")
                    nc.gpsimd.dma_start(
                        out=cp,
                        in_=c_d[m * 128:(m + 1) * 128, q * CB:(q + 1) * CB])
                    t2 = tpool.tile([128, CB], F32, tag="t")
                    nc.vector.scalar_tensor_tensor(
                        t2, tf, mask_sb[:, m:m + 1], cp, ALU.mult, ALU.mult)
                    nc.vector.tensor_add(csl, csl, t2)
                else:  # o
                    to = tpool.tile([128, CB], F32, tag="t")
                    nc.scalar.activation(to, ps, AF.Sigmoid)
                    hh = hpool.tile([128, CB], F32, tag="h")
                    nc.vector.tensor_mul(hh, to, csl)  # csl holds tanh(c_t)
                    nc.sync.dma_start(
                        out=ho_d[m * 128:(m + 1) * 128, q * CB:(q + 1) * CB],
                        in_=hh)

        for gate, q in blocks:
            do_block(gate, q)

        # ---- LayerNorm over H per m-chunk ----
        for m in range(MCH):
            st = tpool.tile([128, 2, 6], F32, tag="st")
            for hf in range(2):
                nc.vector.bn_stats(
                    out=st[:, hf, :], in_=c_acc[:, m, hf * 512:(hf + 1) * 512])
            nc.vector.bn_aggr(out=mvall[:, m, :], in_=st)
        # std = sqrt(var + eps); inv = 1/std; nmi = -mean*inv
        nc.scalar.activation(std_t, mvall[:, :, 1], AF.Sqrt, bias=eps_t)
        nc.vector.reciprocal(inv_t, std_t)
        nc.vector.scalar_tensor_tensor(
            nmi_t, mvall[:, :, 0], -1.0, inv_t, ALU.mult, ALU.mult)
        for m in range(MCH):
            z = zpool.tile([128, H], F32, tag="z")
            nc.scalar.activation(
                z, c_acc[:, m, :], AF.Identity,
                bias=nmi_t[:, m:m + 1], scale=inv_t[:, m:m + 1])
            if not trivial_gb:
                nc.vector.tensor_mul(z, z, gam_bc)
                nc.vector.tensor_add(z, z, bet_bc)
            nc.sync.dma_start(out=co_d[m * 128:(m + 1) * 128, :], in_=z)
            nc.scalar.activation(c_acc[:, m, :], z, AF.Tanh)

        for gate, q in o_blocks:
            do_block(gate, q)

    nc.finalize()
    return nc


def _get_program(trivial_gb: bool):
    if trivial_gb not in _PROGRAMS:
        _PROGRAMS[trivial_gb] = _build_program(trivial_gb)
    return _PROGRAMS[trivial_gb]


def kernel(x, h_prev, c_prev, forget_mask, W, b, ln_gamma, ln_beta):
    from concourse.bass_utils import run_bass_kernel_spmd

    f32 = np.float32
    x = np.ascontiguousarray(x, dtype=f32)
    h_prev = np.ascontiguousarray(h_prev, dtype=f32)
    c_prev = np.ascontiguousarray(c_prev, dtype=f32)
    forget_mask = np.ascontiguousarray(forget_mask, dtype=f32)
    W = np.ascontiguousarray(W, dtype=f32)
    b = np.ascontiguousarray(b, dtype=f32)
    ln_gamma = np.ascontiguousarray(ln_gamma, dtype=f32)
    ln_beta = np.ascontiguousarray(ln_beta, dtype=f32)

    trivial_gb = bool(np.all(ln_gamma == 1.0) and np.all(ln_beta == 0.0))
    nc = _get_program(trivial_gb)

    # pre-transposed [x | h_prev] per shard: [IN+H, BC], contraction-major
    comb_t = np.ascontiguousarray(
        np.concatenate((x, h_prev), axis=1).T)  # [IN+H, B]

    in_maps = []
    for i in range(NCORES):
        sl = slice(i * BC, (i + 1) * BC)
        in_maps.append({
            "combT": np.ascontiguousarray(comb_t[:, sl]),
            "c_prev": c_prev[sl],
            "forget_mask": forget_mask[sl].reshape(MCH, 128),
            "W": W,
            "b": b.reshape(1, 4 * H),
            "ln_gamma": ln_gamma.reshape(1, H),
            "ln_beta": ln_beta.reshape(1, H),
        })

    res = run_bass_kernel_spmd(nc, in_maps, list(range(NCORES)))
    h_t = np.concatenate([r["h_out"] for r in res.results], axis=0)
    c_t = np.concatenate([r["c_out"] for r in res.results], axis=0)
    return (h_t, c_t)
